# revision 58
# baseline (speedup 1.0000x reference)
"""Trainium2 Bass kernel for nn_BasicBlock_HMU (two HMU layers + sync BN + residual).

Sharding: data-parallel over batch (8 cores x 512 rows); mu/lam/v replicated.
BN batch statistics are AllGather-reduced across the 8 cores (sync BN).

Math: all per-n constants in quad (lam_n|mu_n|^2, sum_k (mu.v_k)^2) are
dropped and the cross term -2 sum_k (mu.v_k)(x.v_k) folds into the mu-columns
of the packed weights:
  quad'[b,n] = lam_n*|x_b|^2 + x_b . Wmu_n + sum_k (x_b.v_k)^2
  Wmu_n = -2*lam_n*mu_n - 2*sum_k (mu_n.v_k) v_k           (1024 x 5120 total)
This scales z per column by A_n = exp(c_n/D); BN(A z) with eps'_n = A_n^2*eps
equals BN(z) with eps EXACTLY (z column variance here is ~1e-6, below eps, so
plain scale-invariance would not hold), so the finalize uses a host-built
per-column eps.

Dataflow per layer (batch on partitions, units on free):
  sweep     v-blocks first (q written by Pool reduce of ACT-squared PSUM),
            mu-blocks last (DVE adds PSUM + rank-1 lam|x|^2 into q)
  per bt    exp (ACT, in place on q), z-1 (DVE/Pool), BN stats via ones-
            matmuls accumulating in PSUM across bt, PE-transpose of z-1 into
            zT (per-partition layout, overlaps the collective)
  sync BN   8KB AllGather; gathered stats are strided-DMA'd straight into
            [128, 2*CH, 8] per-partition layout, so the whole finalize is a
            handful of [128, CH]-wide ops; rsqrt = exp(-0.5*ln(var+eps'))
            keeps ACT on a single function-table set
  affine    h^T = A_n * zT + C_n in ONE tensor_scalar (two per-partition
            scalar pointers), split across DVE and Pool
  layer 2   |h|^2 row from squared h^T chunks, emitted interleaved with the
            v-sweep; tail transposes z2 back per bt, fusing the residual add
            with the PSUM read, then stores
"""

import numpy as np

import concourse.bacc as bacc
import concourse.mybir as mybir
import concourse.tile as tile

try:
    from concourse.bass_utils import run_bass_kernel_spmd
except ImportError:  # pragma: no cover
    from bass_utils import run_bass_kernel_spmd

F32 = mybir.dt.float32
F32R = mybir.dt.float32r
BF16 = mybir.dt.bfloat16
Alu = mybir.AluOpType
Act = mybir.ActivationFunctionType

# Restrict the act-table pass to the one set holding every ACT func we use
# (exp, ln, square, copy/identity), so exactly one LoadActFuncSet is emitted
# at kernel start instead of set switches around each Ln.
_orig_gat = bacc.get_activation_tables


def _gat_single_set(arch):
    tabs = _orig_gat(arch)
    if "natural_log_exp_and_others" not in tabs:
        return tabs
    # keep every entry (act_func_set_id is positional) but empty the others
    return {k: (v if k == "natural_log_exp_and_others" else type(v)())
            for k, v in tabs.items()}


bacc.get_activation_tables = _gat_single_set

N_CORES = 8
B, D, N, K = 4096, 1024, 1024, 4
BS = B // N_CORES          # 512 rows per core
NBT = BS // 128            # 4 batch tiles per core
CH = D // 128              # 8 contraction chunks
NB_MU = N // 512           # 2 moving blocks for the mu matmul
NB_V = (N * K) // 512      # 8 moving blocks for the v matmul
NB_TOT = NB_MU + NB_V      # 10
WROWS = D                  # 1024 weight rows (constants folded into Wmu)
WCOLS = N + N * K          # 5120
BN_EPS = 1e-5

_CACHE = {}


def _build_nc(reps=1, loop_reps=0, collectives=True):
    nc = bacc.Bacc("TRN2", target_bir_lowering=False, debug=False,
                   num_devices=N_CORES)

    xT_s = nc.dram_tensor("xT_s", [D, BS], F32, kind="ExternalInput").ap()
    xsq_s = nc.dram_tensor("xsq_s", [1, BS], F32, kind="ExternalInput").ap()
    W1 = nc.dram_tensor("W1", [WROWS, WCOLS], F32, kind="ExternalInput").ap()
    W2 = nc.dram_tensor("W2", [WROWS, WCOLS], F32, kind="ExternalInput").ap()
    lam = nc.dram_tensor("lam", [2, N], F32, kind="ExternalInput").ap()
    gb = nc.dram_tensor("gb", [6, N], F32, kind="ExternalInput").ap()
    cst = nc.dram_tensor("cst", [128, 128], F32, kind="ExternalInput").ap()
    ones_c = nc.dram_tensor("ones_c", [128, 1], F32, kind="ExternalInput").ap()
    out = nc.dram_tensor("out", [BS, N], F32, kind="ExternalOutput").ap()

    with tile.TileContext(nc) as tc:
        with (
            tc.tile_pool(name="const", bufs=1) as constp,
            tc.tile_pool(name="big", bufs=1) as bigp,
            tc.tile_pool(name="wp", bufs=3) as wp,
            tc.tile_pool(name="scr", bufs=2) as scr,
            tc.tile_pool(name="rowp", bufs=1) as rowp,
            tc.tile_pool(name="pmm", bufs=2, space="PSUM") as pmm,
            tc.tile_pool(name="pst", bufs=1, space="PSUM") as pst,
            tc.tile_pool(name="ptr", bufs=2, space="PSUM") as ptr,
            tc.tile_pool(name="dram", bufs=2, space="DRAM") as dramp,
        ):
            # ---- constants / small inputs (loaded inside body, behind the
            # head-critical xt/W DMAs — none is needed before mid-sweep) ----
            ident_r = constp.tile([128, 128], F32R)
            onec_f32r = constp.tile([128, 1], F32R)
            xsqr = constp.tile([1, BS], F32R)          # |x_b|^2 as a row
            lam1t = constp.tile([1, N], F32R)          # lam rows (K=1 rhs)
            lam2t = constp.tile([1, N], F32R)

            def load_consts():
                nc.scalar.dma_start(ident_r[:], cst.bitcast(F32R))
                nc.scalar.dma_start(onec_f32r[:], ones_c.bitcast(F32R))
                nc.scalar.dma_start(xsqr[:], xsq_s.bitcast(F32R))
                nc.scalar.dma_start(lam1t[:], lam[0:1, :].bitcast(F32R))
                nc.scalar.dma_start(lam2t[:], lam[1:2, :].bitcast(F32R))

            # per-n constants in column layout [128, CH], n = c*128 + p
            # (loaded inside body, behind the head-critical xt/W DMAs)
            def col_const(row):
                t = constp.tile([128, CH], F32, tag=f"colc{row}",
                                name=f"colc{row}")
                nc.scalar.dma_start(
                    t[:], gb[row:row + 1, :].rearrange("o (c p) -> (o p) c",
                                                       p=128))
                return t

            # ---- resident big tiles ----
            hT = bigp.tile([128, CH * BS], F32R, tag="hT")    # z1^T -> h^T
            hT2 = bigp.tile([128, CH * BS], F32R, tag="hT2")  # z2^T -> out^T
            hsqr = rowp.tile([1, BS], F32R, tag="hsqr")       # |h_b|^2 row

            def body():
              # rolling weight-block loader: pairs of chunks on the SP HW
              # queue, emitted two blocks ahead so W2 b0 prefetches during
              # the L1 sweep and the BN-sync DMAs find an empty queue
              nb_order = list(range(NB_MU, NB_TOT)) + list(range(NB_MU))
              loads = [(W1, nb) for nb in nb_order] + [(W2, nb) for nb in nb_order]
              w_tiles = []
              li = [0]

              def emit_load():
                  if li[0] >= len(loads):
                      return
                  Wt, nb = loads[li[0]]
                  li[0] += 1
                  w = wp.tile([128, CH * 512], F32R, tag="w", name="w")
                  w3 = w[:].rearrange("p (c f) -> p c f", f=512)
                  for cp in range(CH // 2):
                      nc.sync.dma_start(
                          w3[:, 2 * cp:2 * cp + 2, :],
                          Wt[2 * cp * 128:(2 * cp + 2) * 128,
                             nb * 512:(nb + 1) * 512]
                          .rearrange("(c p) f -> p c f", p=128)
                          .bitcast(F32R))
                  w_tiles.append(w)

              emit_load()                       # W1 block 0 first
              # x^T in two 1MB DMAs on the ACT HW queue
              xt = bigp.tile([128, CH * BS], F32R, tag="xt")
              xt3 = xt[:].rearrange("p (c b) -> p c b", b=BS)
              for half in range(2):
                  nc.scalar.dma_start(
                      xt3[:, half * 4:(half + 1) * 4, :],
                      xT_s[half * 512:(half + 1) * 512, :]
                      .rearrange("(c p) b -> p c b", p=128)
                      .bitcast(F32R))
              emit_load()                       # W1 block 1
              load_consts()
              g1c, b1c = col_const(0), col_const(1)
              g2c, b2c = col_const(2), col_const(3)
              e1c, e2c = col_const(4), col_const(5)
              for L in range(2):
                  W = (W1, W2)[L]
                  lhsT = (xt, hT)[L]
                  sq_row = (xsqr, hsqr)[L]
                  lam_row = (lam1t, lam2t)[L][:]
                  gc = (g1c, g2c)[L]
                  bc = (b1c, b2c)[L]
                  ec = (e1c, e2c)[L]
                  zT = (hT, hT2)[L]

                  q = bigp.tile([128, NBT * N], F32, tag="q")   # quad, then exp
                  z = bigp.tile([128, NBT * N], F32R, tag="z")  # z_m1 = exp(.)-1

                  # stats accumulators (live across the whole post-sweep)
                  ps1 = [pst.tile([1, 512], F32, tag=f"ps1_{h}",
                                  name=f"ps1_{h}") for h in range(2)]
                  ps2 = [pst.tile([1, 512], F32, tag=f"ps2_{h}",
                                  name=f"ps2_{h}") for h in range(2)]
                  if L == 1:
                      ph = pst.tile([1, 512], F32, tag="ps1_0", name="ph")

                  def post_bt_act(bt):
                      """exp, z-1 and z^2 for one batch tile (no PE ops)."""
                      for h2 in range(2):
                          sl = slice(bt * N + h2 * 512, bt * N + (h2 + 1) * 512)
                          nc.scalar.activation(q[:, sl], q[:, sl], Act.Exp,
                                               scale=-1.0 / D)
                          eng = (nc.vector, nc.gpsimd)[h2]
                          eng.tensor_scalar(
                              out=z[:, sl], in0=q[:, sl], scalar1=1.0,
                              scalar2=None, op0=Alu.subtract)

                  # ---- matmul sweep: v blocks first, mu blocks last ----
                  for idx, nb in enumerate(nb_order):
                      is_mu = nb < NB_MU
                      w = w_tiles[L * NB_TOT + idx]
                      emit_load()
                      for bt in range(NBT):
                          pm = pmm.tile([128, 512], F32, tag="pm")
                          for c in range(CH):
                              nc.tensor.matmul(
                                  pm[:],
                                  lhsT[:, c * BS + bt * 128:c * BS + (bt + 1) * 128],
                                  w[:, c * 512:(c + 1) * 512],
                                  start=(c == 0),
                                  stop=(not is_mu and c == CH - 1))
                          if is_mu:
                              # rank-1 lam_n * |x_b|^2 closes the group
                              nc.tensor.matmul(
                                  pm[:], sq_row[:, bt * 128:(bt + 1) * 128],
                                  lam_row[:, nb * 512:(nb + 1) * 512],
                                  start=False, stop=True)
                              ql = q[:, bt * N + nb * 512: bt * N + (nb + 1) * 512]
                              nc.vector.tensor_tensor(
                                  out=ql, in0=pm[:], in1=ql, op=Alu.add)
                              if idx == NB_TOT - 1:
                                  # q[bt] complete: exp/sub can start while the
                                  # PE finishes the remaining mu groups
                                  post_bt_act(bt)
                          else:
                              nv = nb - NB_MU
                              sqv = scr.tile([128, 512], F32, tag="sqv", bufs=3)
                              nc.scalar.activation(sqv[:], pm[:], Act.Square)
                              # sum_k proj^2 written straight into q
                              nc.vector.tensor_reduce(
                                  out=q[:, bt * N + nv * 128: bt * N + (nv + 1) * 128],
                                  in_=sqv[:].rearrange("p (n k) -> p n k", k=K),
                                  axis=mybir.AxisListType.X,
                                  op=Alu.add)
                      if L == 1 and idx < CH:
                          # |h|^2 accumulation, interleaved so the PE reaches
                          # chunk idx long after its square is done
                          hq = scr.tile([128, BS], F32R, tag="hq")
                          nc.scalar.activation(hq[:], hT[:, idx * BS:(idx + 1) * BS],
                                               Act.Square)
                          nc.tensor.matmul(ph[:], onec_f32r[:], hq[:],
                                           start=(idx == 0), stop=(idx == CH - 1))
                          if idx == CH - 1:
                              nc.vector.tensor_copy(hsqr[:], ph[:])

                  # ---- stats matmuls + transposes per bt (PE stream) ----
                  for bt in range(NBT):
                      for h2 in range(2):
                          sl = slice(bt * N + h2 * 512, bt * N + (h2 + 1) * 512)
                          nc.tensor.matmul(ps1[h2][:], onec_f32r[:], z[:, sl],
                                           start=(bt == 0), stop=(bt == NBT - 1))
                          # z^2 as a self-multiply on DVE/Pool so it runs in
                          # parallel with ACT's exps on the collective-critical
                          # chain of the last batch tile
                          zq = scr.tile([128, 512], F32R, tag="zq")
                          eng = (nc.vector, nc.gpsimd)[h2]
                          eng.tensor_tensor(out=zq[:], in0=z[:, sl],
                                            in1=z[:, sl], op=Alu.mult)
                          nc.tensor.matmul(ps2[h2][:], onec_f32r[:], zq[:],
                                           start=(bt == 0), stop=(bt == NBT - 1))
                          # transpose z-1 into per-partition layout
                          pb = ptr.tile([128, 512], F32R, tag="pt")
                          for j in range(4):
                              c = h2 * 4 + j
                              nc.tensor.transpose(
                                  pb[:, j * 128:(j + 1) * 128],
                                  z[:, bt * N + c * 128: bt * N + (c + 1) * 128],
                                  ident_r[:])
                          zT3 = zT[:].rearrange("p (c b) -> p c b", b=BS)
                          nc.scalar.copy(
                              zT3[:, h2 * 4:(h2 + 1) * 4, bt * 128:(bt + 1) * 128],
                              pb[:].rearrange("p (c b) -> p c b", b=128))

                  # ---- sync BN: row stats -> AllGather -> column finalize ----
                  # stats row permuted to m = p*16 + s*8 + c so the
                  # post-gather reload is one conventional 3-dim DMA
                  stats = rowp.tile([1, 2 * N], F32, tag="rows")
                  stats_v = stats[:].rearrange("o (p s c) -> o s c p",
                                               s=2, c=CH)
                  for h2 in range(2):
                      nc.vector.tensor_copy(
                          stats_v[:, 0, h2 * 4:(h2 + 1) * 4, :],
                          ps1[h2][:].rearrange("o (c p) -> o c p", p=128))
                      nc.scalar.copy(
                          stats_v[:, 1, h2 * 4:(h2 + 1) * 4, :],
                          ps2[h2][:].rearrange("o (c p) -> o c p", p=128))
                  cin = dramp.tile([1, 2 * N], F32, tag="cin")
                  nc.sync.dma_start(cin[:], stats[:])
                  cout = dramp.tile([N_CORES, 2 * N], F32, tag="cout",
                                    addr_space="Shared")
                  if collectives:
                      nc.gpsimd.collective_compute(
                          "AllGather", Alu.bypass,
                          replica_groups=[list(range(N_CORES))],
                          ins=[cin[:].opt()], outs=[cout[:].opt()])
                  else:
                      nc.sync.dma_start(cout[0:1, :], cin[:])
                  # reload gathered stats into per-partition layout (one DMA)
                  gT = rowp.tile([128, N_CORES, 2 * CH], F32, tag="gath")
                  nc.sync.dma_start(
                      gT[:], cout[:].rearrange("r (p sc) -> p r sc", p=128))
                  sredc = rowp.tile([128, 2 * CH], F32, tag="sredc")
                  nc.vector.tensor_reduce(
                      out=sredc[:], in_=gT[:].rearrange("p r sc -> p sc r"),
                      axis=mybir.AxisListType.X, op=Alu.add)
                  # finalize, all [128, CH] wide
                  fin = rowp.tile([128, 4 * CH], F32, tag="fincol")
                  mc = fin[:, 0:CH]
                  t0 = fin[:, CH:2 * CH]
                  t1 = fin[:, 2 * CH:3 * CH]
                  ac = rowp.tile([128, CH], F32, tag="Ac")
                  cc = rowp.tile([128, CH], F32, tag="Cc")
                  nc.vector.tensor_scalar(out=mc, in0=sredc[:, 0:CH],
                                          scalar1=1.0 / B, scalar2=None,
                                          op0=Alu.mult)
                  nc.vector.tensor_tensor(out=t0, in0=mc, in1=mc, op=Alu.mult)
                  nc.vector.tensor_tensor(out=t0, in0=t0, in1=ec[:],
                                          op=Alu.subtract)
                  nc.vector.scalar_tensor_tensor(
                      out=t0, in0=sredc[:, CH:2 * CH], scalar=1.0 / B, in1=t0,
                      op0=Alu.mult, op1=Alu.subtract)      # var + eps'
                  nc.scalar.activation(t1, t0, Act.Ln)
                  nc.scalar.activation(t0, t1, Act.Exp, scale=-0.5)  # rsqrt
                  nc.vector.tensor_tensor(out=ac[:], in0=t0, in1=gc[:],
                                          op=Alu.mult)
                  nc.vector.tensor_tensor(out=t1, in0=mc, in1=ac[:], op=Alu.mult)
                  nc.vector.tensor_tensor(out=cc[:], in0=bc[:], in1=t1,
                                          op=Alu.subtract)

                  # ---- affine h^T = A*zT + C, per-partition scalars ----
                  for c in range(CH):
                      eng = (nc.vector, nc.gpsimd)[c % 2]
                      eng.tensor_scalar(
                          out=zT[:, c * BS:(c + 1) * BS],
                          in0=zT[:, c * BS:(c + 1) * BS],
                          scalar1=ac[:, c:c + 1], scalar2=cc[:, c:c + 1],
                          op0=Alu.mult, op1=Alu.add)

                  if L == 1:
                      # ---- tail: transpose back with the residual summed in
                      # PSUM (x block = transpose of the resident x^T block)
                      for bt in range(NBT):
                          ot = scr.tile([128, N], F32, tag="ot", bufs=2)
                          for h2 in range(2):
                              pb = ptr.tile([128, 512], F32R, tag="pt")
                              for j in range(4):
                                  c = h2 * 4 + j
                                  sl = pb[:, j * 128:(j + 1) * 128]
                                  blk = slice(c * BS + bt * 128,
                                              c * BS + (bt + 1) * 128)
                                  nc.tensor.matmul(sl, zT[:, blk], ident_r[:],
                                                   is_transpose=True,
                                                   start=True, stop=False)
                                  nc.tensor.matmul(sl, xt[:, blk], ident_r[:],
                                                   is_transpose=True,
                                                   start=False, stop=True)
                              if h2 == 0:
                                  nc.vector.tensor_copy(
                                      ot[:, 0:512], pb[:].bitcast(F32))
                              else:
                                  nc.scalar.copy(
                                      ot[:, 512:1024], pb[:].bitcast(F32))
                          seng = (nc.scalar, nc.sync)[bt % 2]
                          seng.dma_start(out[bt * 128:(bt + 1) * 128, :],
                                         ot[:])

            if loop_reps:
                with tc.For_i(0, loop_reps, 1):
                    body()
            else:
                for _rep in range(reps):
                    body()

    nc.compile()
    return nc


def _host_prep(x, mu1, lam1, v1, g1, b1, mu2, lam2, v2, g2, b2):
    """Build the device-input arrays (all float32, transposed on host)."""
    def build_w(mu, lam_, v):
        mu64 = mu.astype(np.float64)
        v64 = v.astype(np.float64)
        lam64 = lam_.astype(np.float64)
        # per-n constants in quad are BN-invariant given the eps' repair; the
        # -2 sum_k (mu.v_k)(x.v_k) cross term folds into the mu columns.
        vmu = (v64 * mu64[:, None, :]).sum(-1)                # (n,k)
        wmu = (-2.0 * lam64[:, None] * mu64
               - 2.0 * np.einsum("nk,nkd->nd", vmu, v64))     # (n,d)
        W = np.empty((WROWS, WCOLS), np.float32)
        W[:, :N] = wmu.T.astype(np.float32)
        W[:, N:] = v.reshape(N * K, D).T.astype(np.float32)
        return W

    def eps_adj(mu, lam_, v):
        # dropped consts c_n scale z by A_n = exp(c_n/D); exact BN repair is
        # eps'_n = A_n^2 * eps
        mu64 = mu.astype(np.float64)
        v64 = v.astype(np.float64)
        c = (lam_.astype(np.float64) * (mu64 * mu64).sum(1)
             + ((v64 * mu64[:, None, :]).sum(-1) ** 2).sum(-1))
        return (BN_EPS * np.exp(2.0 * c / D)).astype(np.float32)

    W1 = build_w(mu1, lam1, v1)
    W2 = build_w(mu2, lam2, v2)
    xT = np.ascontiguousarray(x.T)
    xsq = (x.astype(np.float64) ** 2).sum(1).astype(np.float32)
    lam_rows = np.stack([lam1, lam2]).astype(np.float32)
    cst = np.eye(128, dtype=np.float32)
    gb_rows = np.stack([g1, b1, g2, b2,
                        eps_adj(mu1, lam1, v1),
                        eps_adj(mu2, lam2, v2)]).astype(np.float32)

    in_maps = []
    for c in range(N_CORES):
        rs = slice(c * BS, (c + 1) * BS)
        in_maps.append({
            "xT_s": np.ascontiguousarray(xT[:, rs]),
            "xsq_s": np.ascontiguousarray(xsq[rs].reshape(1, BS)),
            "W1": W1, "W2": W2,
            "lam": lam_rows, "gb": gb_rows, "cst": cst,
            "ones_c": np.ones((128, 1), np.float32),
        })
    return in_maps


def kernel(x, mu1, lam1, v1, g1, b1, mu2, lam2, v2, g2, b2):
    if "nc" not in _CACHE:
        _CACHE["nc"] = _build_nc()
    nc = _CACHE["nc"]
    in_maps = _host_prep(x, mu1, lam1, v1, g1, b1, mu2, lam2, v2, g2, b2)
    res = run_bass_kernel_spmd(nc, in_maps, list(range(N_CORES)))
    return np.concatenate([res.results[c]["out"] for c in range(N_CORES)], axis=0)
